# revision 4
# baseline (speedup 1.0000x reference)
"""ARD-RBF kernel matrix on 8 Trainium2 NeuronCores (sacrificial-row fold).

out = variance * exp(-0.5 * (sq1[:,None] + sq2[None,:] - 2*cross))
with alpha = softmax(softplus(alpha_raw)), variance = variance_raw[0]**2,
cross = (x1*alpha) @ x2.T, sq1 = (x1*x1)@alpha, sq2 = (x2*x2)@alpha.

Strategy (rows of x1 sharded 8 ways; x2/alpha/variance replicated):
  - host ships x1.T shard [512,1024] bf16 and x2.T [512,8192] fp8e4m3,
    with the feature dim PERMUTED so the two smallest-alpha features land
    at positions 510,511 (pure layout prep; alpha_raw permuted to match).
  - unnormalized-softmax trick: with u = 1+e^alpha_raw and S = sum(u),
    every alpha-weighted sum is (1/S)*(u-weighted sum).
  - main GEMM in fp8 DoubleRow perf mode; x1a = (u*x1)/4 fp8.
  - COLUMN term folded into the GEMM via sacrificial rows: features
    510,511 (alpha ~ 5e-5, negligible cross contribution) are replaced:
    x1f rows = 1.0; x2f rows = hi/lo fp8 split of T_j = -r2u_j/8 where
    r2u_j = sum_q u_q x2_jq^2.  Then 4*rs*PSUM = rs*cross - 0.5*rs*r2u.
    hi/lo split keeps the column-term error ~0.1%.
  - r2u via M=1 matmuls over DVE-computed x2^2 (squares read the original
    rows 510,511 before they are overwritten; Tile tracks the WAR dep).
  - ROW term + ln(variance) ride the Exp activation bias (per-partition);
    its per-partition scale carries 4*rs.  So the main ACT output IS the
    final bf16 result: no ec2b broadcast, no post-multiply, no gpsimd.
  - column groups [512,1536,2048,2048,2048] so the first PSUM->ACT->DMA
    output flows by ~11us while later groups' prep runs in the shadow.
  - output written bf16 (halves write traffic); host upcasts to f32.
"""

import os
import sys

import numpy as np

sys.path.insert(0, "/opt/trn_rl_repo")

import ml_dtypes

N_CORES = 8
N_ROWS, M_COLS, DIM = 8192, 8192, 512
ROWS = N_ROWS // N_CORES  # 1024 rows of x1 per core
S1 = 4.0                  # x1a fp8 pre-scale (undone via Exp scale)
GROUPS = (512, 1536, 2048, 2048, 2048)   # column group widths


def build_ard_rbf(tc, out, x1t, x2t, araw, vraw, rows, m_cols, dim):
    """Emit the per-core kernel. APs: out [rows, m_cols] bf16,
    x1t [dim, rows] bf16, x2t [dim, m_cols] fp8e4, araw [dim] f32,
    vraw [1] f32."""
    import concourse.mybir as mybir

    nc = tc.nc
    f32 = mybir.dt.float32
    bf16 = mybir.dt.bfloat16
    f8 = mybir.dt.float8e4
    AF = mybir.ActivationFunctionType
    DR = mybir.MatmulPerfMode.DoubleRow

    KC = dim // 128          # contraction chunks (4)
    KP = KC // 2             # DoubleRow chunk pairs (2)
    MT = rows // 128         # output row tiles per core (8)
    NG = len(GROUPS)
    gstart = [sum(GROUPS[:i]) for i in range(NG)]

    with (
        tc.tile_pool(name="const", bufs=1) as const,
        tc.tile_pool(name="x2pool", bufs=1) as x2pool,
        tc.tile_pool(name="sqpool", bufs=1) as sqpool,
        tc.tile_pool(name="work", bufs=2) as work,
        tc.tile_pool(name="outp", bufs=4) as outp,
        tc.tile_pool(name="psum", bufs=2, space="PSUM") as psum,
    ):
        def mainps(name):
            return psum.tile([128, 2048], f32, tag="mainps", name=name)

        # ---------------- constants + HAM warmup first (no deps) -----------
        id1 = const.tile([1, 1], f32)
        nc.vector.memset(id1, 1.0)
        ones128 = const.tile([1, 128], f32)
        nc.vector.memset(ones128, 1.0)
        wones1 = const.tile([1, 128], bf16)
        nc.vector.memset(wones1, 1.0)
        wones5 = const.tile([1, 512], bf16)
        nc.vector.memset(wones5, 1.0)
        warm_ps = mainps("warm_ps")
        for w in range(10):
            nc.tensor.matmul(warm_ps[:, 0:512], lhsT=wones1, rhs=wones5,
                             start=True, stop=True)

        # ---------------- loads: tiny first, g0 cols, x1, rest -------------
        a_row = const.tile([1, dim], f32)
        nc.sync.dma_start(out=a_row, in_=araw.rearrange("(a d) -> a d", a=1))
        vr = const.tile([1, 1], f32)
        nc.sync.dma_start(out=vr, in_=vraw.rearrange("(a d) -> a d", a=1))
        # x2 fp8 in DoubleRow pair layout: x2f[kk][:, j, :] = chunk 2kk+j
        x2f = [
            x2pool.tile([128, 2, m_cols], f8, tag=f"x2f{kk}", name=f"x2f{kk}")
            for kk in range(KP)
        ]

        def load_x2(k, gsl):
            kk, j = divmod(k, 2)
            nc.sync.dma_start(
                out=x2f[kk][:, j : j + 1, gsl],
                in_=x2t[k * 128 : (k + 1) * 128, gsl],
            )

        for k in range(KC):          # first group's columns (0..512)
            load_x2(k, slice(0, GROUPS[0]))
        x1t_c = []
        for k in range(KC):
            xt = const.tile([128, rows], bf16, tag=f"x1t{k}", name=f"x1t_{k}")
            nc.sync.dma_start(out=xt, in_=x1t[k * 128 : (k + 1) * 128, :])
            x1t_c.append(xt)
        for g in range(1, NG):
            for k in range(KC):
                load_x2(k, slice(gstart[g], gstart[g] + GROUPS[g]))

        # ---------------- u = 1 + exp(araw); critical path ------------------
        e0 = const.tile([1, dim], f32)
        sm = const.tile([1, 1], f32)
        nc.scalar.activation(e0, a_row, AF.Exp, accum_out=sm)
        ep_ps = mainps("ep_ps")
        for k in range(KC):
            nc.tensor.transpose(
                ep_ps[:, k : k + 1], e0[:, k * 128 : (k + 1) * 128], id1)

        # squares of x2 for group0 (early: only needs x2-g0 DMA)
        sqx2 = [
            sqpool.tile([128, 2, 2048], bf16, tag=f"sqx2{kk}",
                        name=f"sqx2_{kk}")
            for kk in range(KP)
        ]

        def squares(g, kk, piece=None, npiece=1):
            gsl = slice(gstart[g], gstart[g] + GROUPS[g])
            dsl = slice(0, GROUPS[g])
            if piece is not None:
                w = GROUPS[g] // npiece
                gsl = slice(gstart[g] + piece * w, gstart[g] + (piece + 1) * w)
                dsl = slice(piece * w, (piece + 1) * w)
            nc.vector.tensor_mul(
                sqx2[kk][:, :, dsl], x2f[kk][:, :, gsl], x2f[kk][:, :, gsl])

        for kk in range(KP):
            squares(0, kk)

        with tc.high_priority():
            # u4_p = (1+e)/S1 per chunk column; unegb8 = -(1+e)/8 bf16
            u4_p = const.tile([128, KC], f32)
            nc.vector.tensor_scalar(
                u4_p, ep_ps[:, 0:KC], 1.0 / S1, 1.0 / S1,
                op0=mybir.AluOpType.mult, op1=mybir.AluOpType.add,
            )
            unegb8 = const.tile([128, KC], bf16)
            nc.vector.tensor_scalar(
                unegb8, ep_ps[:, 0:KC], -1.0 / 8.0, -1.0 / 8.0,
                op0=mybir.AluOpType.mult, op1=mybir.AluOpType.add,
            )

            # x1a = (u/S1) * x1, fp8, DoubleRow pair layout (kk=0 first so
            # kk=0 main matmuls can start while kk=1 converts)
            x1f = [
                const.tile([128, 2, rows], f8, tag=f"x1f{kk}", name=f"x1f{kk}")
                for kk in range(KP)
            ]
            for k in range(KC):
                kk, j = divmod(k, 2)
                nc.vector.tensor_scalar_mul(
                    x1f[kk][:, j : j + 1, :], x1t_c[k], u4_p[:, k : k + 1])
            # sacrificial rows: features 510,511 -> constant 1.0
            # (DVE memset can't start at partition 126; DMA a ones row in)
            ones_f8 = const.tile([1, 2 * rows], f8)
            nc.vector.memset(ones_f8, 1.0)
            nc.sync.dma_start(out=x1f[1][126:128, 1:2, :], in_=ones_f8)

        # ---------------- rs, ln(var) broadcast; off critical path ----------
        smd = const.tile([1, 1], f32)
        nc.vector.tensor_scalar_add(smd, sm, float(dim))
        rs = const.tile([1, 1], f32)
        nc.vector.reciprocal(rs, smd)
        lnv = const.tile([1, 1], f32)
        nc.scalar.activation(lnv, vr, AF.Ln)
        # rs_row = [rs, S1*rs, 2*ln(vraw)]; broadcast to [128,3] via K=1 mm
        rs_row = const.tile([1, 3], f32)
        nc.vector.tensor_copy(rs_row[:, 0:1], rs)
        nc.vector.tensor_scalar_mul(rs_row[:, 1:2], rs, S1)
        nc.vector.tensor_scalar_mul(rs_row[:, 2:3], lnv, 2.0)
        rs_ps = mainps("rs_ps")
        nc.tensor.matmul(rs_ps[:, 0:3], lhsT=ones128, rhs=rs_row, start=True,
                         stop=True)
        rs_bc = const.tile([128, 3], f32)
        nc.vector.tensor_copy(rs_bc, rs_ps[:, 0:3])

        # ---------------- colterm for a group: cps -> hi/lo fp8 rows --------
        def colterm_mms(g, ps):
            """M=1 matmuls: ps[0:1, c] = -(1/8) sum_q u_q x2_qc^2."""
            W = GROUPS[g]
            for h in range(W // 512):
                hs = slice(h * 512, (h + 1) * 512)
                for k in range(KC):
                    kk, j = divmod(k, 2)
                    nc.tensor.matmul(
                        ps[0:1, hs],
                        lhsT=unegb8[:, k : k + 1],
                        rhs=sqx2[kk][:, j : j + 1, hs],
                        start=(k == 0), stop=(k == KC - 1),
                    )

        def colterm_rows(g, ps):
            """Split ps row into hi/lo fp8 rows at x2f[1][126:128,1,gcols].
            Small fold to [128, W/128] so DVE ops use many lanes."""
            W = GROUPS[g]
            wf = W // 128
            crow = work.tile([1, 2048], f32, tag="crow", name="crow")
            nc.vector.tensor_copy(crow[:, 0:W], ps[0:1, 0:W])
            cfold = work.tile([128, 16], f32, tag="cfold", name="cfold")
            nc.sync.dma_start(out=cfold[:, 0:wf], in_=crow[:, 0:W])
            h1 = work.tile([128, 16], f8, tag="h1", name="h1")
            nc.vector.tensor_copy(h1[:, 0:wf], cfold[:, 0:wf])
            resid = work.tile([128, 16], f32, tag="resid", name="resid")
            nc.vector.tensor_sub(resid[:, 0:wf], cfold[:, 0:wf], h1[:, 0:wf])
            h2 = work.tile([128, 16], f8, tag="h2", name="h2")
            nc.vector.tensor_copy(h2[:, 0:wf], resid[:, 0:wf])
            gsl = slice(gstart[g], gstart[g] + W)
            nc.sync.dma_start(out=x2f[1][126:127, 1:2, gsl], in_=h1[:, 0:wf])
            nc.sync.dma_start(out=x2f[1][127:128, 1:2, gsl], in_=h2[:, 0:wf])

        cps0 = mainps("cps0")
        colterm_mms(0, cps0)
        colterm_rows(0, cps0)

        # ---------------- r1 = -(1/8)*sum(u*x1^2) on ACT+PE ----------------
        # squares on ScalarE (idle window) to keep DVE free for sq-g1
        r1_ps = mainps("r1_ps")
        sq1t = [
            work.tile([128, 512], bf16, tag=f"sq1_{h}", name=f"sq1_{h}")
            for h in range(2)
        ]
        r1_row = const.tile([1, rows], f32)
        for h in range(rows // 512):
            hs = slice(h * 512, (h + 1) * 512)
            phs = slice(1024 + h * 512, 1024 + (h + 1) * 512)
            for k in range(KC):
                nc.scalar.activation(
                    sq1t[h], x1t_c[k][:, hs], AF.Square)
                kk, j = divmod(k, 2)
                nc.tensor.matmul(
                    r1_ps[0:1, phs], lhsT=unegb8[:, k : k + 1], rhs=sq1t[h],
                    start=(k == 0), stop=(k == KC - 1),
                )
            nc.vector.tensor_copy(r1_row[:, hs], r1_ps[0:1, phs])
        for t in range(MT):
            nc.tensor.transpose(
                r1_ps[:, t : t + 1], r1_row[:, t * 128 : (t + 1) * 128], id1)
        # bias = r1_ps*(S1*rs) + 2*ln(vraw)  (= -0.5*rs*r1u + ln var)
        r1v_t = const.tile([128, MT], f32)
        nc.vector.tensor_scalar(
            r1v_t, r1_ps[:, 0:MT], rs_bc[:, 1:2], rs_bc[:, 2:3],
            op0=mybir.AluOpType.mult, op1=mybir.AluOpType.add,
        )

        # ---------------- main loop over groups x m-tiles -------------------
        # prep for group g+1 distributed across g's m-iterations
        def prep_piece(gn, m):
            if m <= 1:
                squares(gn, m)
            elif m == 2:
                ps = mainps("cps")
                colterm_mms(gn, ps)
                prep_piece.ps = ps
            elif m == 3:
                colterm_rows(gn, prep_piece.ps)

        # persistent [128,4096] output tiles per m; one 1MB DMA (8KB DRAM
        # lines) per (m, half) -- small per-group writes tank DMA efficiency
        ot_tiles = {}
        for m in range(MT):
            ot_tiles[m] = outp.tile([128, 4096], bf16, tag=f"ot{m}", bufs=1,
                                    name=f"ot{m}")
        HALF_END = {2: 0, 4: 1}          # last group index of each half
        for g in range(NG):
            W = GROUPS[g]
            gs = gstart[g]
            half = 0 if g <= 2 else 1
            osl0 = gs - half * 4096      # column offset within ot tile
            for m in range(MT):
                ps = mainps("main")
                msl = slice(m * 128, (m + 1) * 128)
                for kk in range(KP):
                    for h in range(W // 512):
                        sl = slice(gs + h * 512, gs + (h + 1) * 512)
                        nc.tensor.matmul(
                            ps[:, h * 512 : (h + 1) * 512],
                            lhsT=x1f[kk][:, :, msl],
                            rhs=x2f[kk][:, :, sl],
                            start=(kk == 0), stop=(kk == KP - 1),
                            perf_mode=DR,
                        )
                ot = ot_tiles[m]
                nc.scalar.activation(
                    ot[:, osl0 : osl0 + W], ps[:, 0:W], AF.Exp,
                    bias=r1v_t[:, m : m + 1], scale=rs_bc[:, 1:2],
                )
                if g in HALF_END:
                    nc.sync.dma_start(
                        out=out[msl, half * 4096 : (half + 1) * 4096],
                        in_=ot)
                if g + 1 < NG and m <= 3:
                    prep_piece(g + 1, m)


_CACHE = {}


def _get_compiled():
    if "nc" in _CACHE:
        return _CACHE["nc"]
    import concourse.mybir as mybir
    import concourse.tile as tile
    from concourse import bacc

    f32 = mybir.dt.float32
    bf16 = mybir.dt.bfloat16
    f8 = mybir.dt.float8e4
    nc = bacc.Bacc("TRN2", target_bir_lowering=False, debug=False,
                   enable_asserts=False)
    x1t = nc.dram_tensor("x1t", [DIM, ROWS], bf16, kind="ExternalInput").ap()
    x2t = nc.dram_tensor("x2t", [DIM, M_COLS], f8, kind="ExternalInput").ap()
    araw = nc.dram_tensor("alpha_raw", [DIM], f32, kind="ExternalInput").ap()
    vraw = nc.dram_tensor("variance_raw", [1], f32, kind="ExternalInput").ap()
    out = nc.dram_tensor("out", [ROWS, M_COLS], bf16,
                         kind="ExternalOutput").ap()

    with tile.TileContext(nc) as tc:
        build_ard_rbf(tc, out, x1t, x2t, araw, vraw, ROWS, M_COLS, DIM)
    nc.compile()
    _CACHE["nc"] = nc
    return nc


def kernel(x1, x2, alpha_raw, variance_raw):
    from concourse import bass_utils

    x1 = np.asarray(x1, dtype=np.float32)
    x2 = np.asarray(x2, dtype=np.float32)
    alpha_raw = np.ascontiguousarray(np.asarray(alpha_raw, dtype=np.float32))
    variance_raw = np.ascontiguousarray(
        np.asarray(variance_raw, dtype=np.float32))

    # permute features so the two smallest-alpha features sit at 510,511
    # (their cross contribution is dropped by the sacrificial rows)
    order = np.argsort(alpha_raw)
    perm = np.concatenate([np.sort(order[2:]), order[1::-1]])
    x1p = x1[:, perm]
    x2p = x2[:, perm]
    alpha_p = np.ascontiguousarray(alpha_raw[perm])

    x1t_full = np.ascontiguousarray(x1p.T).astype(ml_dtypes.bfloat16)
    x2t_full = np.ascontiguousarray(x2p.T).astype(ml_dtypes.float8_e4m3)

    nc = _get_compiled()
    in_maps = []
    for c in range(N_CORES):
        in_maps.append({
            "x1t": np.ascontiguousarray(x1t_full[:, c * ROWS : (c + 1) * ROWS]),
            "x2t": x2t_full,
            "alpha_raw": alpha_p,
            "variance_raw": variance_raw,
        })
    res = bass_utils.run_bass_kernel_spmd(
        nc, in_maps, core_ids=list(range(N_CORES)),
        trace=bool(int(os.environ.get("ARD_TRACE", "0"))),
        tmpdir=os.environ.get("ARD_TMPDIR"),
    )
    _CACHE["last_results"] = res
    out = np.concatenate(
        [res.results[c]["out"] for c in range(N_CORES)], axis=0)
    return out.astype(np.float32)


if __name__ == "__main__":
    rng = np.random.default_rng(0)
    ins = {
        "x1": rng.standard_normal((N_ROWS, DIM), dtype=np.float32),
        "x2": rng.standard_normal((M_COLS, DIM), dtype=np.float32),
        "alpha_raw": rng.standard_normal((DIM,), dtype=np.float32),
        "variance_raw": rng.random((1,), dtype=np.float32),
    }
    o = kernel(**ins)
    print(o.shape, o.dtype)


# revision 5
# speedup vs baseline: 1.0734x; 1.0734x over previous
"""ARD-RBF kernel matrix on 8 Trainium2 NeuronCores (sacrificial-row fold).

out = variance * exp(-0.5 * (sq1[:,None] + sq2[None,:] - 2*cross))
with alpha = softmax(softplus(alpha_raw)), variance = variance_raw[0]**2,
cross = (x1*alpha) @ x2.T, sq1 = (x1*x1)@alpha, sq2 = (x2*x2)@alpha.

Strategy (rows of x1 sharded 8 ways; x2/alpha/variance replicated):
  - host ships x1.T shard [512,1024] bf16 and x2.T [512,8192] fp8e4m3,
    with the feature dim PERMUTED so the two smallest-alpha features land
    at positions 510,511 (pure layout prep; alpha_raw permuted to match).
  - unnormalized-softmax trick: with u = 1+e^alpha_raw and S = sum(u),
    every alpha-weighted sum is (1/S)*(u-weighted sum).
  - main GEMM in fp8 DoubleRow perf mode; x1a = (u*x1)/4 fp8.
  - COLUMN term folded into the GEMM via sacrificial rows: features
    510,511 (alpha ~ 5e-5, negligible cross contribution) are replaced:
    x1f rows = 1.0; x2f rows = hi/lo fp8 split of T_j = -r2u_j/8 where
    r2u_j = sum_q u_q x2_jq^2.  Then 4*rs*PSUM = rs*cross - 0.5*rs*r2u.
  - r2u via M=1 DoubleRow matmuls over fp8 x2^2 tiles (squares read the
    original rows 510,511 before overwrite; Tile tracks the WAR dep).
    Colterm gets its OWN psum banks so the main ping-pong stays 2-deep.
  - ROW term + ln(variance) ride the Exp activation bias (per-partition);
    scale carries 4*rs.  Main ACT output IS the final bf16 result: no
    broadcast, no post-multiply, no gpsimd.
  - column groups [512,1536x5]; main psum tiles [128,1536] (3 banks x2)
    + colterm [128,512] (1 bank x2) = 8 banks exactly.
  - output written bf16 via persistent [128,4608] ot tiles, one DMA per
    (m, half) with 7-9KB DRAM lines; host upcasts to f32.
"""

import os
import sys

import numpy as np

sys.path.insert(0, "/opt/trn_rl_repo")

import ml_dtypes

N_CORES = 8
N_ROWS, M_COLS, DIM = 8192, 8192, 512
ROWS = N_ROWS // N_CORES  # 1024 rows of x1 per core
S1 = 4.0                  # x1a fp8 pre-scale (undone via Exp scale)
GROUPS = (512, 1536, 1536, 1536, 1536, 1536)   # column group widths
HALF_END = {2: (0, 3584), 5: (3584, 8192)}     # g -> output col range


def build_ard_rbf(tc, out, x1t, x2t, araw, vraw, rows, m_cols, dim):
    """Emit the per-core kernel. APs: out [rows, m_cols] bf16,
    x1t [dim, rows] bf16, x2t [dim, m_cols] fp8e4, araw [dim] f32,
    vraw [1] f32."""
    import concourse.mybir as mybir

    nc = tc.nc
    f32 = mybir.dt.float32
    bf16 = mybir.dt.bfloat16
    f8 = mybir.dt.float8e4
    AF = mybir.ActivationFunctionType
    DR = mybir.MatmulPerfMode.DoubleRow

    KC = dim // 128          # contraction chunks (4)
    KP = KC // 2             # DoubleRow chunk pairs (2)
    MT = rows // 128         # output row tiles per core (8)
    NG = len(GROUPS)
    gstart = [sum(GROUPS[:i]) for i in range(NG)]

    with (
        tc.tile_pool(name="const", bufs=1) as const,
        tc.tile_pool(name="x2pool", bufs=1) as x2pool,
        tc.tile_pool(name="sqpool", bufs=1) as sqpool,
        tc.tile_pool(name="work", bufs=2) as work,
        tc.tile_pool(name="outp", bufs=1) as outp,
        tc.tile_pool(name="psum", bufs=2, space="PSUM") as psum,
    ):
        def mainps(name):
            return psum.tile([128, 1536], f32, tag="mainps", name=name)

        def ctps(name):
            return psum.tile([128, 512], f32, tag="ctps", name=name)

        # ---------------- constants + HAM warmup first (no deps) -----------
        id1 = const.tile([1, 1], f32)
        nc.vector.memset(id1, 1.0)
        ones128 = const.tile([1, 128], f32)
        nc.vector.memset(ones128, 1.0)
        wones1 = const.tile([1, 128], bf16)
        nc.vector.memset(wones1, 1.0)
        wones5 = const.tile([1, 512], bf16)
        nc.vector.memset(wones5, 1.0)
        warm_ps = mainps("warm_ps")
        for w in range(10):
            nc.tensor.matmul(warm_ps[:, 0:512], lhsT=wones1, rhs=wones5,
                             start=True, stop=True)

        # ---------------- loads: tiny first, g0 cols, x1, rest -------------
        a_row = const.tile([1, dim], f32)
        nc.sync.dma_start(out=a_row, in_=araw.rearrange("(a d) -> a d", a=1))
        vr = const.tile([1, 1], f32)
        nc.sync.dma_start(out=vr, in_=vraw.rearrange("(a d) -> a d", a=1))
        # x2 fp8 in DoubleRow pair layout: x2f[kk][:, j, :] = chunk 2kk+j
        x2f = [
            x2pool.tile([128, 2, m_cols], f8, tag=f"x2f{kk}", name=f"x2f{kk}")
            for kk in range(KP)
        ]

        def load_x2(k, gsl):
            kk, j = divmod(k, 2)
            nc.sync.dma_start(
                out=x2f[kk][:, j : j + 1, gsl],
                in_=x2t[k * 128 : (k + 1) * 128, gsl],
            )

        for k in range(KC):          # first group's columns (0..512)
            load_x2(k, slice(0, GROUPS[0]))
        x1t_c = []
        for k in range(KC):
            xt = const.tile([128, rows], bf16, tag=f"x1t{k}", name=f"x1t_{k}")
            nc.sync.dma_start(out=xt, in_=x1t[k * 128 : (k + 1) * 128, :])
            x1t_c.append(xt)
        for g in range(1, NG):
            for k in range(KC):
                load_x2(k, slice(gstart[g], gstart[g] + GROUPS[g]))

        # ---------------- u = 1 + exp(araw); critical path ------------------
        e0 = const.tile([1, dim], f32)
        sm = const.tile([1, 1], f32)
        nc.scalar.activation(e0, a_row, AF.Exp, accum_out=sm)
        ep_ps = mainps("ep_ps")
        for k in range(KC):
            nc.tensor.transpose(
                ep_ps[:, k : k + 1], e0[:, k * 128 : (k + 1) * 128], id1)

        # squares of x2 (fp8) for group0 (early: only needs x2-g0 DMA)
        sqx2 = [
            sqpool.tile([128, 2, 1536], f8, tag=f"sqx2{kk}",
                        name=f"sqx2_{kk}")
            for kk in range(KP)
        ]

        def squares(g, kk):
            gsl = slice(gstart[g], gstart[g] + GROUPS[g])
            dsl = slice(0, GROUPS[g])
            nc.vector.tensor_mul(
                sqx2[kk][:, :, dsl], x2f[kk][:, :, gsl], x2f[kk][:, :, gsl])

        for kk in range(KP):
            squares(0, kk)

        with tc.high_priority():
            # u4_p = (1+e)/S1; une8f[:, j, kk] = -(1+e_{2kk+j})/8 fp8 weights
            u4_p = const.tile([128, KC], f32)
            nc.vector.tensor_scalar(
                u4_p, ep_ps[:, 0:KC], 1.0 / S1, 1.0 / S1,
                op0=mybir.AluOpType.mult, op1=mybir.AluOpType.add,
            )
            une8f = const.tile([128, 2, 16], f8)   # dim1 stride 16 (DR rule)
            for k in range(KC):
                kk, j = divmod(k, 2)
                nc.vector.tensor_scalar(
                    une8f[:, j : j + 1, kk : kk + 1], ep_ps[:, k : k + 1],
                    -1.0 / 8.0, -1.0 / 8.0,
                    op0=mybir.AluOpType.mult, op1=mybir.AluOpType.add,
                )

            # x1a = (u/S1) * x1, fp8, DoubleRow pair layout (kk=0 first)
            x1f = [
                const.tile([128, 2, rows], f8, tag=f"x1f{kk}", name=f"x1f{kk}")
                for kk in range(KP)
            ]
            for k in range(KC):
                kk, j = divmod(k, 2)
                nc.vector.tensor_scalar_mul(
                    x1f[kk][:, j : j + 1, :], x1t_c[k], u4_p[:, k : k + 1])
            # sacrificial rows: features 510,511 -> constant 1.0
            # (DVE can't start at partition 126; DMA a ones row in)
            ones_f8 = const.tile([1, 2 * rows], f8)
            nc.vector.memset(ones_f8, 1.0)
            nc.sync.dma_start(out=x1f[1][126:128, 1:2, :], in_=ones_f8)

        # ---------------- rs, ln(var) broadcast; off critical path ----------
        smd = const.tile([1, 1], f32)
        nc.vector.tensor_scalar_add(smd, sm, float(dim))
        rs = const.tile([1, 1], f32)
        nc.vector.reciprocal(rs, smd)
        lnv = const.tile([1, 1], f32)
        nc.scalar.activation(lnv, vr, AF.Ln)
        # rs_row = [rs, S1*rs, 2*ln(vraw)]; broadcast to [128,3] via K=1 mm
        rs_row = const.tile([1, 3], f32)
        nc.vector.tensor_copy(rs_row[:, 0:1], rs)
        nc.vector.tensor_scalar_mul(rs_row[:, 1:2], rs, S1)
        nc.vector.tensor_scalar_mul(rs_row[:, 2:3], lnv, 2.0)
        rs_ps = mainps("rs_ps")
        nc.tensor.matmul(rs_ps[:, 0:3], lhsT=ones128, rhs=rs_row, start=True,
                         stop=True)
        rs_bc = const.tile([128, 3], f32)
        nc.vector.tensor_copy(rs_bc, rs_ps[:, 0:3])

        # ---------------- colterm: cps chunks -> crow -> hi/lo fp8 rows -----
        def colterm_chunk(g, c, crow):
            """ctps chunk: -(1/8) sum_q u_q x2_qc^2 for 512 cols, DR fp8,
            then copy into crow[0, c*512:...]. Own psum tag: never touches
            the main ping-pong."""
            ps = ctps(f"ct_{g}_{c}")
            hs = slice(c * 512, (c + 1) * 512)
            for kk in range(KP):
                nc.tensor.matmul(
                    ps[0:1, 0:512],
                    lhsT=une8f[:, :, kk : kk + 1],
                    rhs=sqx2[kk][:, :, hs],
                    start=(kk == 0), stop=(kk == KP - 1),
                    perf_mode=DR,
                )
            nc.vector.tensor_copy(crow[:, hs], ps[0:1, 0:512])

        def colterm_rows(g, crow):
            """Split crow into hi/lo fp8 rows at x2f[1][126:128,1,gcols].
            Small fold to [128, W/128] so DVE ops use many lanes."""
            W = GROUPS[g]
            wf = W // 128
            cfold = work.tile([128, 16], f32, tag="cfold", name="cfold")
            nc.sync.dma_start(out=cfold[:, 0:wf], in_=crow[:, 0:W])
            h1 = work.tile([128, 16], f8, tag="h1", name="h1")
            nc.vector.tensor_copy(h1[:, 0:wf], cfold[:, 0:wf])
            resid = work.tile([128, 16], f32, tag="resid", name="resid")
            nc.vector.tensor_sub(resid[:, 0:wf], cfold[:, 0:wf], h1[:, 0:wf])
            h2 = work.tile([128, 16], f8, tag="h2", name="h2")
            nc.vector.tensor_copy(h2[:, 0:wf], resid[:, 0:wf])
            gsl = slice(gstart[g], gstart[g] + W)
            nc.sync.dma_start(out=x2f[1][126:127, 1:2, gsl], in_=h1[:, 0:wf])
            nc.sync.dma_start(out=x2f[1][127:128, 1:2, gsl], in_=h2[:, 0:wf])

        crow0 = work.tile([1, 1536], f32, tag="crow", name="crow0")
        colterm_chunk(0, 0, crow0)
        colterm_rows(0, crow0)

        # ---------------- r1 = -(1/8)*sum(u*x1^2) on ACT+PE ----------------
        # squares on ScalarE (its idle window) to keep DVE free
        r1_ps = mainps("r1_ps")
        sq1t = [
            work.tile([128, 512], bf16, tag=f"sq1_{h}", name=f"sq1_{h}")
            for h in range(2)
        ]
        une_b = const.tile([128, KC], bf16)
        nc.vector.tensor_scalar(
            une_b, ep_ps[:, 0:KC], -1.0 / 8.0, -1.0 / 8.0,
            op0=mybir.AluOpType.mult, op1=mybir.AluOpType.add,
        )
        r1_row = const.tile([1, rows], f32)
        for h in range(rows // 512):
            hs = slice(h * 512, (h + 1) * 512)
            phs = slice(512 + h * 512, 512 + (h + 1) * 512)
            for k in range(KC):
                nc.scalar.activation(sq1t[h], x1t_c[k][:, hs], AF.Square)
                nc.tensor.matmul(
                    r1_ps[0:1, phs], lhsT=une_b[:, k : k + 1], rhs=sq1t[h],
                    start=(k == 0), stop=(k == KC - 1),
                )
            nc.vector.tensor_copy(r1_row[:, hs], r1_ps[0:1, phs])
        for t in range(MT):
            nc.tensor.transpose(
                r1_ps[:, t : t + 1], r1_row[:, t * 128 : (t + 1) * 128], id1)
        # bias = r1_ps*(S1*rs) + 2*ln(vraw)  (= -0.5*rs*r1u + ln var)
        r1v_t = const.tile([128, MT], f32)
        nc.vector.tensor_scalar(
            r1v_t, r1_ps[:, 0:MT], rs_bc[:, 1:2], rs_bc[:, 2:3],
            op0=mybir.AluOpType.mult, op1=mybir.AluOpType.add,
        )

        # ---------------- main loop over groups x m-tiles -------------------
        # prep for group g+1 distributed across g's m-iterations
        def prep_piece(gn, m):
            if m <= 1:
                squares(gn, m)
            elif m == 2:
                prep_piece.crow = work.tile([1, 1536], f32, tag="crow",
                                            name=f"crow{gn}")
            if 2 <= m <= 4:
                c = m - 2
                if c < GROUPS[gn] // 512:
                    colterm_chunk(gn, c, prep_piece.crow)
            elif m == 5:
                colterm_rows(gn, prep_piece.crow)

        # persistent output tiles per m; one DMA per (m, half):
        # half0 = cols 0:3584 (7KB lines), half1 = 3584:8192 (9KB lines)
        ot_tiles = {}
        for m in range(MT):
            ot_tiles[m] = outp.tile([128, 4608], bf16, tag=f"ot{m}", bufs=1,
                                    name=f"ot{m}")
        for g in range(NG):
            W = GROUPS[g]
            gs = gstart[g]
            osl0 = gs - (3584 if g > 2 else 0)
            for m in range(MT):
                ps = mainps("main")
                msl = slice(m * 128, (m + 1) * 128)
                for kk in range(KP):
                    for h in range(W // 512):
                        sl = slice(gs + h * 512, gs + (h + 1) * 512)
                        nc.tensor.matmul(
                            ps[:, h * 512 : (h + 1) * 512],
                            lhsT=x1f[kk][:, :, msl],
                            rhs=x2f[kk][:, :, sl],
                            start=(kk == 0), stop=(kk == KP - 1),
                            perf_mode=DR,
                        )
                ot = ot_tiles[m]
                nc.scalar.activation(
                    ot[:, osl0 : osl0 + W], ps[:, 0:W], AF.Exp,
                    bias=r1v_t[:, m : m + 1], scale=rs_bc[:, 1:2],
                )
                if g in HALF_END:
                    c0, c1 = HALF_END[g]
                    nc.sync.dma_start(
                        out=out[msl, c0:c1], in_=ot[:, 0 : c1 - c0])
                if g + 1 < NG and m <= 5:
                    prep_piece(g + 1, m)


_CACHE = {}


def _get_compiled():
    if "nc" in _CACHE:
        return _CACHE["nc"]
    import concourse.mybir as mybir
    import concourse.tile as tile
    from concourse import bacc

    f32 = mybir.dt.float32
    bf16 = mybir.dt.bfloat16
    f8 = mybir.dt.float8e4
    nc = bacc.Bacc("TRN2", target_bir_lowering=False, debug=False,
                   enable_asserts=False)
    x1t = nc.dram_tensor("x1t", [DIM, ROWS], bf16, kind="ExternalInput").ap()
    x2t = nc.dram_tensor("x2t", [DIM, M_COLS], f8, kind="ExternalInput").ap()
    araw = nc.dram_tensor("alpha_raw", [DIM], f32, kind="ExternalInput").ap()
    vraw = nc.dram_tensor("variance_raw", [1], f32, kind="ExternalInput").ap()
    out = nc.dram_tensor("out", [ROWS, M_COLS], bf16,
                         kind="ExternalOutput").ap()

    with tile.TileContext(nc) as tc:
        build_ard_rbf(tc, out, x1t, x2t, araw, vraw, ROWS, M_COLS, DIM)
    nc.compile()
    _CACHE["nc"] = nc
    return nc


def kernel(x1, x2, alpha_raw, variance_raw):
    from concourse import bass_utils

    x1 = np.asarray(x1, dtype=np.float32)
    x2 = np.asarray(x2, dtype=np.float32)
    alpha_raw = np.ascontiguousarray(np.asarray(alpha_raw, dtype=np.float32))
    variance_raw = np.ascontiguousarray(
        np.asarray(variance_raw, dtype=np.float32))

    # permute features so the two smallest-alpha features sit at 510,511
    # (their cross contribution is dropped by the sacrificial rows)
    order = np.argsort(alpha_raw)
    perm = np.concatenate([np.sort(order[2:]), order[1::-1]])
    x1p = x1[:, perm]
    x2p = x2[:, perm]
    alpha_p = np.ascontiguousarray(alpha_raw[perm])

    x1t_full = np.ascontiguousarray(x1p.T).astype(ml_dtypes.bfloat16)
    x2t_full = np.ascontiguousarray(x2p.T).astype(ml_dtypes.float8_e4m3)

    nc = _get_compiled()
    in_maps = []
    for c in range(N_CORES):
        in_maps.append({
            "x1t": np.ascontiguousarray(x1t_full[:, c * ROWS : (c + 1) * ROWS]),
            "x2t": x2t_full,
            "alpha_raw": alpha_p,
            "variance_raw": variance_raw,
        })
    res = bass_utils.run_bass_kernel_spmd(
        nc, in_maps, core_ids=list(range(N_CORES)),
        trace=bool(int(os.environ.get("ARD_TRACE", "0"))),
        tmpdir=os.environ.get("ARD_TMPDIR"),
    )
    _CACHE["last_results"] = res
    out = np.concatenate(
        [res.results[c]["out"] for c in range(N_CORES)], axis=0)
    return out.astype(np.float32)


if __name__ == "__main__":
    rng = np.random.default_rng(0)
    ins = {
        "x1": rng.standard_normal((N_ROWS, DIM), dtype=np.float32),
        "x2": rng.standard_normal((M_COLS, DIM), dtype=np.float32),
        "alpha_raw": rng.standard_normal((DIM,), dtype=np.float32),
        "variance_raw": rng.random((1,), dtype=np.float32),
    }
    o = kernel(**ins)
    print(o.shape, o.dtype)


# revision 7
# speedup vs baseline: 1.3000x; 1.2111x over previous
"""ARD-RBF kernel matrix on 8 Trainium2 NeuronCores (sacrificial-row fold).

out = variance * exp(-0.5 * (sq1[:,None] + sq2[None,:] - 2*cross))
with alpha = softmax(softplus(alpha_raw)), variance = variance_raw[0]**2,
cross = (x1*alpha) @ x2.T, sq1 = (x1*x1)@alpha, sq2 = (x2*x2)@alpha.

Strategy (rows of x1 sharded 8 ways; x2/alpha/variance replicated):
  - host ships x1.T shard [512,1024] bf16 and x2.T [512,8192] fp8e4m3,
    with the feature dim PERMUTED so the two smallest-alpha features land
    at positions 510,511 (pure layout prep; alpha_raw permuted to match).
  - unnormalized-softmax trick: with u = 1+e^alpha_raw and S = sum(u),
    every alpha-weighted sum is (1/S)*(u-weighted sum).
  - main GEMM in fp8 DoubleRow perf mode; x1a = (u*x1)/4 fp8.
  - COLUMN term folded into the GEMM via sacrificial rows: features
    510,511 (alpha ~ 5e-5, negligible cross contribution) are replaced:
    x1f rows = 1.0; x2f rows = hi/lo fp8 split of T_j = -r2u_j/8 where
    r2u_j = sum_q u_q x2_jq^2.  Then 4*rs*PSUM = rs*cross - 0.5*rs*r2u.
  - r2u via M=1 DoubleRow matmuls over fp8 x2^2 tiles (squares read the
    original rows 510,511 before overwrite; Tile tracks the WAR dep).
    Colterm gets its OWN psum banks so the main ping-pong stays 2-deep.
  - ROW term + ln(variance) ride the Exp activation bias (per-partition);
    scale carries 4*rs.  Main ACT output IS the final bf16 result: no
    broadcast, no post-multiply, no gpsimd.
  - column groups [512,1536x5]; main psum tiles [128,1536] (3 banks x2)
    + colterm [128,512] (1 bank x2) = 8 banks exactly.
  - output written bf16 via persistent [128,4608] ot tiles, one DMA per
    (m, half) with 7-9KB DRAM lines; host upcasts to f32.
"""

import os
import sys

import numpy as np

sys.path.insert(0, "/opt/trn_rl_repo")

import ml_dtypes

N_CORES = 8
N_ROWS, M_COLS, DIM = 8192, 8192, 512
ROWS = N_ROWS // N_CORES  # 1024 rows of x1 per core
S1 = 4.0                  # x1a fp8 pre-scale (undone via Exp scale)
GROUPS = (512, 1536, 1536, 1536, 1536, 1536)   # column group widths
# g -> (out col range, ot col range): thirds keep the DMA queue drained
OUT_AT = {2: (0, 3584, 0, 3584), 4: (3584, 6656, 0, 3072),
          5: (6656, 8192, 3072, 4608)}


def build_ard_rbf(tc, out, x1t, x2t, araw, vraw, rows, m_cols, dim):
    """Emit the per-core kernel. APs: out [rows, m_cols] bf16,
    x1t [dim, rows] bf16, x2t [dim, m_cols] fp8e4, araw [dim] f32,
    vraw [1] f32."""
    import concourse.mybir as mybir

    nc = tc.nc
    f32 = mybir.dt.float32
    bf16 = mybir.dt.bfloat16
    f8 = mybir.dt.float8e4
    AF = mybir.ActivationFunctionType
    DR = mybir.MatmulPerfMode.DoubleRow

    KC = dim // 128          # contraction chunks (4)
    KP = KC // 2             # DoubleRow chunk pairs (2)
    MT = rows // 128         # output row tiles per core (8)
    NG = len(GROUPS)
    gstart = [sum(GROUPS[:i]) for i in range(NG)]

    with (
        tc.tile_pool(name="const", bufs=1) as const,
        tc.tile_pool(name="x2pool", bufs=1) as x2pool,
        tc.tile_pool(name="sqpool", bufs=1) as sqpool,
        tc.tile_pool(name="work", bufs=2) as work,
        tc.tile_pool(name="outp", bufs=1) as outp,
        tc.tile_pool(name="psum", bufs=2, space="PSUM") as psum,
    ):
        def mainps(name):
            return psum.tile([128, 1536], f32, tag="mainps", name=name)

        def ctps(name):
            return psum.tile([128, 512], f32, tag="ctps", name=name)

        # ---------------- constants + HAM warmup first (no deps) -----------
        id1 = const.tile([1, 1], f32)
        nc.vector.memset(id1, 1.0)
        ones128 = const.tile([1, 128], f32)
        nc.vector.memset(ones128, 1.0)
        wones1 = const.tile([1, 128], bf16)
        nc.vector.memset(wones1, 1.0)
        wones5 = const.tile([1, 512], bf16)
        nc.vector.memset(wones5, 1.0)
        warm_ps = mainps("warm_ps")
        for w in range(10):
            nc.tensor.matmul(warm_ps[:, 0:512], lhsT=wones1, rhs=wones5,
                             start=True, stop=True)

        # ---------------- loads: tiny first, g0 cols, x1, rest -------------
        a_row = const.tile([1, dim], f32)
        nc.sync.dma_start(out=a_row, in_=araw.rearrange("(a d) -> a d", a=1))
        vr = const.tile([1, 1], f32)
        nc.sync.dma_start(out=vr, in_=vraw.rearrange("(a d) -> a d", a=1))
        # x2 fp8 in DoubleRow pair layout: x2f[kk][:, j, :] = chunk 2kk+j
        x2f = [
            x2pool.tile([128, 2, m_cols], f8, tag=f"x2f{kk}", name=f"x2f{kk}")
            for kk in range(KP)
        ]

        # consolidated loads (the Sync engine serializes DMA triggers at
        # ~0.7us each -- keep the count low): one DMA per (kk, col-range)
        def load_x2(kk, gsl):
            nc.sync.dma_start(
                out=x2f[kk][:, :, gsl],
                in_=x2t[256 * kk : 256 * kk + 256, gsl].rearrange(
                    "(j p) c -> p j c", j=2),
            )

        for kk in range(KP):         # first group's columns (0..512)
            load_x2(kk, slice(0, GROUPS[0]))
        x1t_all = const.tile([128, KC, rows], bf16, name="x1t_all")
        nc.sync.dma_start(
            out=x1t_all,
            in_=x1t.rearrange("(c p) r -> p c r", p=128))
        x1t_c = [x1t_all[:, k, :] for k in range(KC)]
        for kk in range(KP):         # group 1
            load_x2(kk, slice(gstart[1], gstart[1] + GROUPS[1]))
        for kk in range(KP):         # groups 2..5 in one shot
            load_x2(kk, slice(gstart[2], m_cols))

        # ---------------- u = 1 + exp(araw); critical path ------------------
        e0 = const.tile([1, dim], f32)
        sm = const.tile([1, 1], f32)
        nc.scalar.activation(e0, a_row, AF.Exp, accum_out=sm)
        ep_ps = mainps("ep_ps")
        for k in range(KC):
            nc.tensor.transpose(
                ep_ps[:, k : k + 1], e0[:, k * 128 : (k + 1) * 128], id1)

        # squares of x2 (fp8) for group0 (early: only needs x2-g0 DMA)
        sqx2 = [
            sqpool.tile([128, 2, 1536], f8, tag=f"sqx2{kk}",
                        name=f"sqx2_{kk}")
            for kk in range(KP)
        ]

        def squares(g, kk):
            gsl = slice(gstart[g], gstart[g] + GROUPS[g])
            dsl = slice(0, GROUPS[g])
            nc.vector.tensor_mul(
                sqx2[kk][:, :, dsl], x2f[kk][:, :, gsl], x2f[kk][:, :, gsl])

        for kk in range(KP):
            squares(0, kk)

        with tc.high_priority():
            # u4_p = (1+e)/S1; une8f[:, j, kk] = -(1+e_{2kk+j})/8 fp8 weights
            u4_p = const.tile([128, KC], f32)
            nc.vector.tensor_scalar(
                u4_p, ep_ps[:, 0:KC], 1.0 / S1, 1.0 / S1,
                op0=mybir.AluOpType.mult, op1=mybir.AluOpType.add,
            )
            une8f = const.tile([128, 2, 16], f8)   # dim1 stride 16 (DR rule)
            for k in range(KC):
                kk, j = divmod(k, 2)
                nc.vector.tensor_scalar(
                    une8f[:, j : j + 1, kk : kk + 1], ep_ps[:, k : k + 1],
                    -1.0 / 8.0, -1.0 / 8.0,
                    op0=mybir.AluOpType.mult, op1=mybir.AluOpType.add,
                )

            # x1a = (u/S1) * x1, fp8, DoubleRow pair layout (kk=0 first)
            x1f = [
                const.tile([128, 2, rows], f8, tag=f"x1f{kk}", name=f"x1f{kk}")
                for kk in range(KP)
            ]
            for k in range(KC):
                kk, j = divmod(k, 2)
                nc.vector.tensor_scalar_mul(
                    x1f[kk][:, j : j + 1, :], x1t_c[k], u4_p[:, k : k + 1])
            # sacrificial rows: features 510,511 -> constant 1.0
            # (DVE can't start at partition 126; DMA a ones row in)
            ones_f8 = const.tile([1, 2 * rows], f8)
            nc.vector.memset(ones_f8, 1.0)
            nc.gpsimd.dma_start(out=x1f[1][126:128, 1:2, :],
                                in_=ones_f8)

        # ---------------- rs, ln(var) broadcast; off critical path ----------
        smd = const.tile([1, 1], f32)
        nc.vector.tensor_scalar_add(smd, sm, float(dim))
        rs = const.tile([1, 1], f32)
        nc.vector.reciprocal(rs, smd)
        lnv = const.tile([1, 1], f32)
        nc.scalar.activation(lnv, vr, AF.Ln)
        # rs_row = [rs, S1*rs, 2*ln(vraw)]; broadcast to [128,3] via K=1 mm
        rs_row = const.tile([1, 3], f32)
        nc.vector.tensor_copy(rs_row[:, 0:1], rs)
        nc.vector.tensor_scalar_mul(rs_row[:, 1:2], rs, S1)
        nc.vector.tensor_scalar_mul(rs_row[:, 2:3], lnv, 2.0)
        rs_ps = mainps("rs_ps")
        nc.tensor.matmul(rs_ps[:, 0:3], lhsT=ones128, rhs=rs_row, start=True,
                         stop=True)
        rs_bc = const.tile([128, 3], f32)
        nc.vector.tensor_copy(rs_bc, rs_ps[:, 0:3])

        # ---------------- colterm: cps chunks -> crow -> hi/lo fp8 rows -----
        def colterm_chunk(g, c, crow):
            """ctps chunk: -(1/8) sum_q u_q x2_qc^2 for 512 cols, DR fp8,
            then copy into crow[0, c*512:...]. Own psum tag: never touches
            the main ping-pong."""
            ps = ctps(f"ct_{g}_{c}")
            hs = slice(c * 512, (c + 1) * 512)
            for kk in range(KP):
                nc.tensor.matmul(
                    ps[0:1, 0:512],
                    lhsT=une8f[:, :, kk : kk + 1],
                    rhs=sqx2[kk][:, :, hs],
                    start=(kk == 0), stop=(kk == KP - 1),
                    perf_mode=DR,
                )
            nc.vector.tensor_copy(crow[:, hs], ps[0:1, 0:512])

        def colterm_rows(g, crow):
            """Split crow into hi/lo fp8 rows at x2f[1][126:128,1,gcols].
            Small fold to [128, W/128] so DVE ops use many lanes."""
            W = GROUPS[g]
            wf = W // 128
            cfold = work.tile([128, 16], f32, tag="cfold", name="cfold")
            nc.gpsimd.dma_start(out=cfold[:, 0:wf], in_=crow[:, 0:W])
            h1 = work.tile([128, 16], f8, tag="h1", name="h1")
            nc.vector.tensor_copy(h1[:, 0:wf], cfold[:, 0:wf])
            resid = work.tile([128, 16], f32, tag="resid", name="resid")
            nc.vector.tensor_sub(resid[:, 0:wf], cfold[:, 0:wf], h1[:, 0:wf])
            h2 = work.tile([128, 16], f8, tag="h2", name="h2")
            nc.vector.tensor_copy(h2[:, 0:wf], resid[:, 0:wf])
            gsl = slice(gstart[g], gstart[g] + W)
            nc.gpsimd.dma_start(out=x2f[1][126:127, 1:2, gsl],
                                in_=h1[:, 0:wf])
            nc.gpsimd.dma_start(out=x2f[1][127:128, 1:2, gsl],
                                in_=h2[:, 0:wf])

        crow0 = work.tile([1, 1536], f32, tag="crow", name="crow0")
        colterm_chunk(0, 0, crow0)
        colterm_rows(0, crow0)

        # ---------------- r1 = -(1/8)*sum(u*x1^2) on ACT+PE ----------------
        # squares on ScalarE (its idle window) to keep DVE free
        r1_ps = mainps("r1_ps")
        sq1t = [
            work.tile([128, 512], bf16, tag=f"sq1_{h}", name=f"sq1_{h}")
            for h in range(2)
        ]
        une_b = const.tile([128, KC], bf16)
        nc.vector.tensor_scalar(
            une_b, ep_ps[:, 0:KC], -1.0 / 8.0, -1.0 / 8.0,
            op0=mybir.AluOpType.mult, op1=mybir.AluOpType.add,
        )
        r1_row = const.tile([1, rows], f32)
        for h in range(rows // 512):
            hs = slice(h * 512, (h + 1) * 512)
            phs = slice(512 + h * 512, 512 + (h + 1) * 512)
            for k in range(KC):
                nc.scalar.activation(sq1t[h], x1t_c[k][:, hs], AF.Square)
                nc.tensor.matmul(
                    r1_ps[0:1, phs], lhsT=une_b[:, k : k + 1], rhs=sq1t[h],
                    start=(k == 0), stop=(k == KC - 1),
                )
            nc.vector.tensor_copy(r1_row[:, hs], r1_ps[0:1, phs])
        for t in range(MT):
            nc.tensor.transpose(
                r1_ps[:, t : t + 1], r1_row[:, t * 128 : (t + 1) * 128], id1)
        # bias = r1_ps*(S1*rs) + 2*ln(vraw)  (= -0.5*rs*r1u + ln var)
        r1v_t = const.tile([128, MT], f32)
        nc.vector.tensor_scalar(
            r1v_t, r1_ps[:, 0:MT], rs_bc[:, 1:2], rs_bc[:, 2:3],
            op0=mybir.AluOpType.mult, op1=mybir.AluOpType.add,
        )

        # ---------------- main loop over groups x m-tiles -------------------
        # prep for group g+1 distributed across g's m-iterations
        def prep_piece(gn, m):
            if m <= 1:
                squares(gn, m)
            elif m == 2:
                prep_piece.crow = work.tile([1, 1536], f32, tag="crow",
                                            name=f"crow{gn}")
            if 2 <= m <= 4:
                c = m - 2
                if c < GROUPS[gn] // 512:
                    colterm_chunk(gn, c, prep_piece.crow)
            elif m == 5:
                colterm_rows(gn, prep_piece.crow)

        # persistent output tiles per m; one DMA per (m, half):
        # half0 = cols 0:3584 (7KB lines), half1 = 3584:8192 (9KB lines)
        ot_tiles = {}
        for m in range(MT):
            ot_tiles[m] = outp.tile([128, 4608], bf16, tag=f"ot{m}", bufs=1,
                                    name=f"ot{m}")
        for g in range(NG):
            W = GROUPS[g]
            gs = gstart[g]
            osl0 = gs - (3584 if g > 2 else 0)
            for m in range(MT):
                ps = mainps("main")
                msl = slice(m * 128, (m + 1) * 128)
                for kk in range(KP):
                    for h in range(W // 512):
                        sl = slice(gs + h * 512, gs + (h + 1) * 512)
                        nc.tensor.matmul(
                            ps[:, h * 512 : (h + 1) * 512],
                            lhsT=x1f[kk][:, :, msl],
                            rhs=x2f[kk][:, :, sl],
                            start=(kk == 0), stop=(kk == KP - 1),
                            perf_mode=DR,
                        )
                ot = ot_tiles[m]
                nc.scalar.activation(
                    ot[:, osl0 : osl0 + W], ps[:, 0:W], AF.Exp,
                    bias=r1v_t[:, m : m + 1], scale=rs_bc[:, 1:2],
                )
                if g in OUT_AT:
                    c0, c1, t0, t1 = OUT_AT[g]
                    nc.sync.dma_start(
                        out=out[msl, c0:c1], in_=ot[:, t0:t1])
                if g + 1 < NG and m <= 5:
                    prep_piece(g + 1, m)


_CACHE = {}


def _get_compiled():
    if "nc" in _CACHE:
        return _CACHE["nc"]
    import concourse.mybir as mybir
    import concourse.tile as tile
    from concourse import bacc

    f32 = mybir.dt.float32
    bf16 = mybir.dt.bfloat16
    f8 = mybir.dt.float8e4
    nc = bacc.Bacc("TRN2", target_bir_lowering=False, debug=False,
                   enable_asserts=False)
    x1t = nc.dram_tensor("x1t", [DIM, ROWS], bf16, kind="ExternalInput").ap()
    x2t = nc.dram_tensor("x2t", [DIM, M_COLS], f8, kind="ExternalInput").ap()
    araw = nc.dram_tensor("alpha_raw", [DIM], f32, kind="ExternalInput").ap()
    vraw = nc.dram_tensor("variance_raw", [1], f32, kind="ExternalInput").ap()
    out = nc.dram_tensor("out", [ROWS, M_COLS], bf16,
                         kind="ExternalOutput").ap()

    with tile.TileContext(nc) as tc:
        build_ard_rbf(tc, out, x1t, x2t, araw, vraw, ROWS, M_COLS, DIM)
    nc.compile()
    _CACHE["nc"] = nc
    return nc


def kernel(x1, x2, alpha_raw, variance_raw):
    from concourse import bass_utils

    x1 = np.asarray(x1, dtype=np.float32)
    x2 = np.asarray(x2, dtype=np.float32)
    alpha_raw = np.ascontiguousarray(np.asarray(alpha_raw, dtype=np.float32))
    variance_raw = np.ascontiguousarray(
        np.asarray(variance_raw, dtype=np.float32))

    # permute features so the two smallest-alpha features sit at 510,511
    # (their cross contribution is dropped by the sacrificial rows)
    order = np.argsort(alpha_raw)
    perm = np.concatenate([np.sort(order[2:]), order[1::-1]])
    x1p = x1[:, perm]
    x2p = x2[:, perm]
    alpha_p = np.ascontiguousarray(alpha_raw[perm])

    x1t_full = np.ascontiguousarray(x1p.T).astype(ml_dtypes.bfloat16)
    x2t_full = np.ascontiguousarray(x2p.T).astype(ml_dtypes.float8_e4m3)

    nc = _get_compiled()
    in_maps = []
    for c in range(N_CORES):
        in_maps.append({
            "x1t": np.ascontiguousarray(x1t_full[:, c * ROWS : (c + 1) * ROWS]),
            "x2t": x2t_full,
            "alpha_raw": alpha_p,
            "variance_raw": variance_raw,
        })
    res = bass_utils.run_bass_kernel_spmd(
        nc, in_maps, core_ids=list(range(N_CORES)),
        trace=bool(int(os.environ.get("ARD_TRACE", "0"))),
        tmpdir=os.environ.get("ARD_TMPDIR"),
    )
    _CACHE["last_results"] = res
    out = np.concatenate(
        [res.results[c]["out"] for c in range(N_CORES)], axis=0)
    return out.astype(np.float32)


if __name__ == "__main__":
    rng = np.random.default_rng(0)
    ins = {
        "x1": rng.standard_normal((N_ROWS, DIM), dtype=np.float32),
        "x2": rng.standard_normal((M_COLS, DIM), dtype=np.float32),
        "alpha_raw": rng.standard_normal((DIM,), dtype=np.float32),
        "variance_raw": rng.random((1,), dtype=np.float32),
    }
    o = kernel(**ins)
    print(o.shape, o.dtype)


# revision 8
# speedup vs baseline: 1.3509x; 1.0391x over previous
"""ARD-RBF kernel matrix on 8 Trainium2 NeuronCores (sacrificial-row fold).

out = variance * exp(-0.5 * (sq1[:,None] + sq2[None,:] - 2*cross))
with alpha = softmax(softplus(alpha_raw)), variance = variance_raw[0]**2,
cross = (x1*alpha) @ x2.T, sq1 = (x1*x1)@alpha, sq2 = (x2*x2)@alpha.

Strategy (rows of x1 sharded 8 ways; x2/alpha/variance replicated):
  - host ships x1.T shard [512,1024] bf16 and x2.T [512,8192] fp8e4m3,
    with the feature dim PERMUTED so the two smallest-alpha features land
    at positions 510,511 (pure layout prep; alpha_raw permuted to match).
  - unnormalized-softmax trick: with u = 1+e^alpha_raw and S = sum(u),
    every alpha-weighted sum is (1/S)*(u-weighted sum).
  - main GEMM in fp8 DoubleRow perf mode; x1a = (u*x1)/4 fp8.
  - COLUMN term folded into the GEMM via sacrificial rows: features
    510,511 (alpha ~ 5e-5, negligible cross contribution) are replaced:
    x1f rows = 1.0; x2f rows = hi/lo fp8 split of T_j = -r2u_j/8 where
    r2u_j = sum_q u_q x2_jq^2.  Then 4*rs*PSUM = rs*cross - 0.5*rs*r2u.
  - r2u via M=1 DoubleRow matmuls over fp8 x2^2 tiles (squares read the
    original rows 510,511 before overwrite; Tile tracks the WAR dep).
    Colterm gets its OWN psum banks so the main ping-pong stays 2-deep.
  - ROW term + ln(variance) ride the Exp activation bias (per-partition);
    scale carries 4*rs.  Main ACT output IS the final bf16 result: no
    broadcast, no post-multiply, no gpsimd.
  - column groups [512,1536x5]; main psum tiles [128,1536] (3 banks x2)
    + colterm [128,512] (1 bank x2) = 8 banks exactly.
  - output written bf16 via persistent [128,4608] ot tiles, one DMA per
    (m, half) with 7-9KB DRAM lines; host upcasts to f32.
"""

import os
import sys

import numpy as np

sys.path.insert(0, "/opt/trn_rl_repo")

import ml_dtypes

N_CORES = 8
N_ROWS, M_COLS, DIM = 8192, 8192, 512
ROWS = N_ROWS // N_CORES  # 1024 rows of x1 per core
S1 = 4.0                  # x1a fp8 pre-scale (undone via Exp scale)
GROUPS = (512, 1536, 1536, 1536, 1536, 1536)   # column group widths
# g -> (out col range, ot col range): per-group writes keep the DMA
# queue drained all run long (g0's 512 cols ride g1's write)
OUT_AT = {1: (0, 2048, 0, 2048), 2: (2048, 3584, 2048, 3584),
          3: (3584, 5120, 0, 1536), 4: (5120, 6656, 1536, 3072),
          5: (6656, 8192, 3072, 4608)}


def build_ard_rbf(tc, out, x1t, x2t, araw, vraw, rows, m_cols, dim):
    """Emit the per-core kernel. APs: out [rows, m_cols] bf16,
    x1t [dim, rows] bf16, x2t [dim, m_cols] fp8e4, araw [dim] f32,
    vraw [1] f32."""
    import concourse.mybir as mybir

    nc = tc.nc
    f32 = mybir.dt.float32
    bf16 = mybir.dt.bfloat16
    f8 = mybir.dt.float8e4
    AF = mybir.ActivationFunctionType
    DR = mybir.MatmulPerfMode.DoubleRow

    KC = dim // 128          # contraction chunks (4)
    KP = KC // 2             # DoubleRow chunk pairs (2)
    MT = rows // 128         # output row tiles per core (8)
    NG = len(GROUPS)
    gstart = [sum(GROUPS[:i]) for i in range(NG)]

    with (
        tc.tile_pool(name="const", bufs=1) as const,
        tc.tile_pool(name="x2pool", bufs=1) as x2pool,
        tc.tile_pool(name="sqpool", bufs=1) as sqpool,
        tc.tile_pool(name="work", bufs=2) as work,
        tc.tile_pool(name="outp", bufs=1) as outp,
        tc.tile_pool(name="psum", bufs=2, space="PSUM") as psum,
    ):
        def mainps(name):
            return psum.tile([128, 1536], f32, tag="mainps", name=name)

        def ctps(name):
            return psum.tile([128, 512], f32, tag="ctps", name=name)

        # ---------------- constants + HAM warmup first (no deps) -----------
        id1 = const.tile([1, 1], f32)
        nc.vector.memset(id1, 1.0)
        ones128 = const.tile([1, 128], f32)
        nc.vector.memset(ones128, 1.0)
        wones1 = const.tile([1, 128], bf16)
        nc.vector.memset(wones1, 1.0)
        wones5 = const.tile([1, 512], bf16)
        nc.vector.memset(wones5, 1.0)
        warm_ps = mainps("warm_ps")
        for w in range(10):
            nc.tensor.matmul(warm_ps[:, 0:512], lhsT=wones1, rhs=wones5,
                             start=True, stop=True)

        # ---------------- loads: tiny first, g0 cols, x1, rest -------------
        a_row = const.tile([1, dim], f32)
        nc.sync.dma_start(out=a_row, in_=araw.rearrange("(a d) -> a d", a=1))
        vr = const.tile([1, 1], f32)
        nc.sync.dma_start(out=vr, in_=vraw.rearrange("(a d) -> a d", a=1))
        # x2 fp8 in DoubleRow pair layout: x2f[kk][:, j, :] = chunk 2kk+j
        x2f = [
            x2pool.tile([128, 2, m_cols], f8, tag=f"x2f{kk}", name=f"x2f{kk}")
            for kk in range(KP)
        ]

        # consolidated loads (the Sync engine serializes DMA triggers at
        # ~0.7us each -- keep the count low): one DMA per (kk, col-range)
        def load_x2(kk, gsl):
            nc.sync.dma_start(
                out=x2f[kk][:, :, gsl],
                in_=x2t[256 * kk : 256 * kk + 256, gsl].rearrange(
                    "(j p) c -> p j c", j=2),
            )

        for kk in range(KP):         # first group's columns (0..512)
            load_x2(kk, slice(0, GROUPS[0]))
        x1t_all = const.tile([128, KC, rows], bf16, name="x1t_all")
        nc.sync.dma_start(
            out=x1t_all,
            in_=x1t.rearrange("(c p) r -> p c r", p=128))
        x1t_c = [x1t_all[:, k, :] for k in range(KC)]
        for kk in range(KP):         # group 1
            load_x2(kk, slice(gstart[1], gstart[1] + GROUPS[1]))
        for kk in range(KP):         # group 2
            load_x2(kk, slice(gstart[2], gstart[2] + GROUPS[2]))
        for kk in range(KP):         # groups 3..5 in one shot
            load_x2(kk, slice(gstart[3], m_cols))

        # ---------------- u = 1 + exp(araw); critical path ------------------
        e0 = const.tile([1, dim], f32)
        sm = const.tile([1, 1], f32)
        nc.scalar.activation(e0, a_row, AF.Exp, accum_out=sm)
        ep_ps = mainps("ep_ps")
        for k in range(KC):
            nc.tensor.transpose(
                ep_ps[:, k : k + 1], e0[:, k * 128 : (k + 1) * 128], id1)

        # squares of x2 (fp8) for group0 (early: only needs x2-g0 DMA)
        sqx2 = [
            sqpool.tile([128, 2, 1536], f8, tag=f"sqx2{kk}",
                        name=f"sqx2_{kk}")
            for kk in range(KP)
        ]

        def squares(g, kk):
            gsl = slice(gstart[g], gstart[g] + GROUPS[g])
            dsl = slice(0, GROUPS[g])
            nc.vector.tensor_mul(
                sqx2[kk][:, :, dsl], x2f[kk][:, :, gsl], x2f[kk][:, :, gsl])

        for kk in range(KP):
            squares(0, kk)

        with tc.high_priority():
            # u4_p = (1+e)/S1; une8f[:, j, kk] = -(1+e_{2kk+j})/8 fp8 weights
            u4_p = const.tile([128, KC], f32)
            nc.vector.tensor_scalar(
                u4_p, ep_ps[:, 0:KC], 1.0 / S1, 1.0 / S1,
                op0=mybir.AluOpType.mult, op1=mybir.AluOpType.add,
            )
            une8f = const.tile([128, 2, 16], f8)   # dim1 stride 16 (DR rule)
            for k in range(KC):
                kk, j = divmod(k, 2)
                nc.vector.tensor_scalar(
                    une8f[:, j : j + 1, kk : kk + 1], ep_ps[:, k : k + 1],
                    -1.0 / 8.0, -1.0 / 8.0,
                    op0=mybir.AluOpType.mult, op1=mybir.AluOpType.add,
                )

            # x1a = (u/S1) * x1, fp8, DoubleRow pair layout (kk=0 first)
            x1f = [
                const.tile([128, 2, rows], f8, tag=f"x1f{kk}", name=f"x1f{kk}")
                for kk in range(KP)
            ]
            for k in range(KC):
                kk, j = divmod(k, 2)
                nc.scalar.activation(
                    x1f[kk][:, j : j + 1, :], x1t_c[k], AF.Copy,
                    scale=u4_p[:, k : k + 1])
            # sacrificial rows: features 510,511 -> constant 1.0
            # (DVE can't start at partition 126; DMA a ones row in)
            ones_f8 = const.tile([1, 2 * rows], f8)
            nc.vector.memset(ones_f8, 1.0)
            nc.gpsimd.dma_start(out=x1f[1][126:128, 1:2, :],
                                in_=ones_f8)

        # ---------------- rs, ln(var) broadcast; off critical path ----------
        smd = const.tile([1, 1], f32)
        nc.vector.tensor_scalar_add(smd, sm, float(dim))
        rs = const.tile([1, 1], f32)
        nc.vector.reciprocal(rs, smd)
        lnv = const.tile([1, 1], f32)
        nc.scalar.activation(lnv, vr, AF.Ln)
        # rs_row = [rs, S1*rs, 2*ln(vraw)]; broadcast to [128,3] via K=1 mm
        rs_row = const.tile([1, 3], f32)
        nc.vector.tensor_copy(rs_row[:, 0:1], rs)
        nc.vector.tensor_scalar_mul(rs_row[:, 1:2], rs, S1)
        nc.vector.tensor_scalar_mul(rs_row[:, 2:3], lnv, 2.0)
        rs_ps = mainps("rs_ps")
        nc.tensor.matmul(rs_ps[:, 0:3], lhsT=ones128, rhs=rs_row, start=True,
                         stop=True)
        rs_bc = const.tile([128, 3], f32)
        nc.vector.tensor_copy(rs_bc, rs_ps[:, 0:3])

        # ---------------- colterm: cps chunks -> crow -> hi/lo fp8 rows -----
        def colterm_chunk(g, c, crow):
            """ctps chunk: -(1/8) sum_q u_q x2_qc^2 for 512 cols, DR fp8,
            then copy into crow[0, c*512:...]. Own psum tag: never touches
            the main ping-pong."""
            ps = ctps(f"ct_{g}_{c}")
            hs = slice(c * 512, (c + 1) * 512)
            for kk in range(KP):
                nc.tensor.matmul(
                    ps[0:1, 0:512],
                    lhsT=une8f[:, :, kk : kk + 1],
                    rhs=sqx2[kk][:, :, hs],
                    start=(kk == 0), stop=(kk == KP - 1),
                    perf_mode=DR,
                )
            nc.vector.tensor_copy(crow[:, hs], ps[0:1, 0:512])

        def colterm_rows(g, crow):
            """Split crow into hi/lo fp8 rows at x2f[1][126:128,1,gcols].
            Small fold to [128, W/128] so DVE ops use many lanes."""
            W = GROUPS[g]
            wf = W // 128
            cfold = work.tile([128, 16], f32, tag="cfold", name="cfold")
            nc.gpsimd.dma_start(out=cfold[:, 0:wf], in_=crow[:, 0:W])
            h1 = work.tile([128, 16], f8, tag="h1", name="h1")
            nc.vector.tensor_copy(h1[:, 0:wf], cfold[:, 0:wf])
            resid = work.tile([128, 16], f32, tag="resid", name="resid")
            nc.vector.tensor_sub(resid[:, 0:wf], cfold[:, 0:wf], h1[:, 0:wf])
            h2 = work.tile([128, 16], f8, tag="h2", name="h2")
            nc.vector.tensor_copy(h2[:, 0:wf], resid[:, 0:wf])
            gsl = slice(gstart[g], gstart[g] + W)
            nc.gpsimd.dma_start(out=x2f[1][126:127, 1:2, gsl],
                                in_=h1[:, 0:wf])
            nc.gpsimd.dma_start(out=x2f[1][127:128, 1:2, gsl],
                                in_=h2[:, 0:wf])

        crow0 = work.tile([1, 1536], f32, tag="crow", name="crow0")
        colterm_chunk(0, 0, crow0)
        colterm_rows(0, crow0)

        # ---------------- r1 = -(1/8)*sum(u*x1^2) on ACT+PE ----------------
        # squares on ScalarE (its idle window) to keep DVE free
        r1_ps = mainps("r1_ps")
        sq1t = [
            work.tile([128, 512], bf16, tag=f"sq1_{h}", name=f"sq1_{h}")
            for h in range(2)
        ]
        une_b = const.tile([128, KC], bf16)
        nc.vector.tensor_scalar(
            une_b, ep_ps[:, 0:KC], -1.0 / 8.0, -1.0 / 8.0,
            op0=mybir.AluOpType.mult, op1=mybir.AluOpType.add,
        )
        r1_row = const.tile([1, rows], f32)
        for h in range(rows // 512):
            hs = slice(h * 512, (h + 1) * 512)
            phs = slice(512 + h * 512, 512 + (h + 1) * 512)
            for k in range(KC):
                nc.vector.tensor_mul(sq1t[h], x1t_c[k][:, hs],
                                     x1t_c[k][:, hs])
                nc.tensor.matmul(
                    r1_ps[0:1, phs], lhsT=une_b[:, k : k + 1], rhs=sq1t[h],
                    start=(k == 0), stop=(k == KC - 1),
                )
            nc.vector.tensor_copy(r1_row[:, hs], r1_ps[0:1, phs])
        for t in range(MT):
            nc.tensor.transpose(
                r1_ps[:, t : t + 1], r1_row[:, t * 128 : (t + 1) * 128], id1)
        # bias = r1_ps*(S1*rs) + 2*ln(vraw)  (= -0.5*rs*r1u + ln var)
        r1v_t = const.tile([128, MT], f32)
        nc.vector.tensor_scalar(
            r1v_t, r1_ps[:, 0:MT], rs_bc[:, 1:2], rs_bc[:, 2:3],
            op0=mybir.AluOpType.mult, op1=mybir.AluOpType.add,
        )

        # ---------------- main loop over groups x m-tiles -------------------
        # prep for group g+1 distributed across g's m-iterations
        def prep_piece(gn, m):
            if m <= 1:
                squares(gn, m)
            elif m == 2:
                prep_piece.crow = work.tile([1, 1536], f32, tag="crow",
                                            name=f"crow{gn}")
            if 2 <= m <= 4:
                c = m - 2
                if c < GROUPS[gn] // 512:
                    colterm_chunk(gn, c, prep_piece.crow)
            elif m == 5:
                colterm_rows(gn, prep_piece.crow)

        # persistent output tiles per m; one DMA per (m, half):
        # half0 = cols 0:3584 (7KB lines), half1 = 3584:8192 (9KB lines)
        ot_tiles = {}
        for m in range(MT):
            ot_tiles[m] = outp.tile([128, 4608], bf16, tag=f"ot{m}", bufs=1,
                                    name=f"ot{m}")
        for g in range(NG):
            W = GROUPS[g]
            gs = gstart[g]
            osl0 = gs - (3584 if g > 2 else 0)
            for m in range(MT):
                ps = mainps("main")
                msl = slice(m * 128, (m + 1) * 128)
                for kk in range(KP):
                    for h in range(W // 512):
                        sl = slice(gs + h * 512, gs + (h + 1) * 512)
                        nc.tensor.matmul(
                            ps[:, h * 512 : (h + 1) * 512],
                            lhsT=x1f[kk][:, :, msl],
                            rhs=x2f[kk][:, :, sl],
                            start=(kk == 0), stop=(kk == KP - 1),
                            perf_mode=DR,
                        )
                ot = ot_tiles[m]
                nc.scalar.activation(
                    ot[:, osl0 : osl0 + W], ps[:, 0:W], AF.Exp,
                    bias=r1v_t[:, m : m + 1], scale=rs_bc[:, 1:2],
                )
                if g in OUT_AT:
                    c0, c1, t0, t1 = OUT_AT[g]
                    nc.sync.dma_start(
                        out=out[msl, c0:c1], in_=ot[:, t0:t1])
                if g + 1 < NG and m <= 5:
                    prep_piece(g + 1, m)


_CACHE = {}


def _get_compiled():
    if "nc" in _CACHE:
        return _CACHE["nc"]
    import concourse.mybir as mybir
    import concourse.tile as tile
    from concourse import bacc

    f32 = mybir.dt.float32
    bf16 = mybir.dt.bfloat16
    f8 = mybir.dt.float8e4
    nc = bacc.Bacc("TRN2", target_bir_lowering=False, debug=False,
                   enable_asserts=False)
    x1t = nc.dram_tensor("x1t", [DIM, ROWS], bf16, kind="ExternalInput").ap()
    x2t = nc.dram_tensor("x2t", [DIM, M_COLS], f8, kind="ExternalInput").ap()
    araw = nc.dram_tensor("alpha_raw", [DIM], f32, kind="ExternalInput").ap()
    vraw = nc.dram_tensor("variance_raw", [1], f32, kind="ExternalInput").ap()
    out = nc.dram_tensor("out", [ROWS, M_COLS], bf16,
                         kind="ExternalOutput").ap()

    with tile.TileContext(nc) as tc:
        build_ard_rbf(tc, out, x1t, x2t, araw, vraw, ROWS, M_COLS, DIM)
    nc.compile()
    _CACHE["nc"] = nc
    return nc


def kernel(x1, x2, alpha_raw, variance_raw):
    from concourse import bass_utils

    x1 = np.asarray(x1, dtype=np.float32)
    x2 = np.asarray(x2, dtype=np.float32)
    alpha_raw = np.ascontiguousarray(np.asarray(alpha_raw, dtype=np.float32))
    variance_raw = np.ascontiguousarray(
        np.asarray(variance_raw, dtype=np.float32))

    # permute features so the two smallest-alpha features sit at 510,511
    # (their cross contribution is dropped by the sacrificial rows)
    order = np.argsort(alpha_raw)
    perm = np.concatenate([np.sort(order[2:]), order[1::-1]])
    x1p = x1[:, perm]
    x2p = x2[:, perm]
    alpha_p = np.ascontiguousarray(alpha_raw[perm])

    x1t_full = np.ascontiguousarray(x1p.T).astype(ml_dtypes.bfloat16)
    x2t_full = np.ascontiguousarray(x2p.T).astype(ml_dtypes.float8_e4m3)

    nc = _get_compiled()
    in_maps = []
    for c in range(N_CORES):
        in_maps.append({
            "x1t": np.ascontiguousarray(x1t_full[:, c * ROWS : (c + 1) * ROWS]),
            "x2t": x2t_full,
            "alpha_raw": alpha_p,
            "variance_raw": variance_raw,
        })
    res = bass_utils.run_bass_kernel_spmd(
        nc, in_maps, core_ids=list(range(N_CORES)),
        trace=bool(int(os.environ.get("ARD_TRACE", "0"))),
        tmpdir=os.environ.get("ARD_TMPDIR"),
    )
    _CACHE["last_results"] = res
    out = np.concatenate(
        [res.results[c]["out"] for c in range(N_CORES)], axis=0)
    return out.astype(np.float32)


if __name__ == "__main__":
    rng = np.random.default_rng(0)
    ins = {
        "x1": rng.standard_normal((N_ROWS, DIM), dtype=np.float32),
        "x2": rng.standard_normal((M_COLS, DIM), dtype=np.float32),
        "alpha_raw": rng.standard_normal((DIM,), dtype=np.float32),
        "variance_raw": rng.random((1,), dtype=np.float32),
    }
    o = kernel(**ins)
    print(o.shape, o.dtype)
